# revision 1
# baseline (speedup 1.0000x reference)
"""Distributed TRN2 Bass kernel for causal multi-head attention
(B=2, L=2048, D=1024, H=16, HD=64) on 8 NeuronCores.

Sharding: tensor-parallel over heads — 2 heads per core, full sequence on
every core. The output projection contracts over all 16 heads, so per-head
attention outputs are exchanged with a single 8-core AllToAll that
simultaneously re-shards by sequence block: core r ends up with all heads
for sequence rows [r*512, (r+1)*512) and computes that slice of the output
projection (plus bias). The host concatenates the 8 slices.

Precision: matmuls run in float32r (TF32-like: fp32 with 11-bit mantissa,
1 PE cycle/row vs 4 for fp32). Host pre-rounds all DMA-fed matmul operands
to the f32r grid (round-to-nearest-even, 12 low mantissa bits dropped) so
no on-device rounding passes are needed. Softmax runs without max
subtraction (scores are bounded; exp stays in fp32 range) with the
denominator computed by an extra all-ones column in the V stationary.
"""
import numpy as np

import concourse.bass as bass
import concourse.tile as tile
from concourse import bacc, mybir
from concourse.bass_utils import run_bass_kernel_spmd

# problem shape (hardcoded per harness contract)
B, L, D = 2, 2048, 1024
H, HD = 16, 64
BL = B * L                      # 4096
N_CORES = 8
H_PER = H // N_CORES            # 2 heads per core
EV = H_PER * HD                 # 128: packed per-core head dim
SEQ_SH = BL // N_CORES          # 512: output rows per core after A2A

F32 = mybir.dt.float32
F32R = mybir.dt.float32r
BF16 = mybir.dt.bfloat16
CHUNK = 256                     # moving-dim chunk for projection/rope
N_CH = BL // CHUNK              # 8
KT = 128                        # kpos tile
N_KT = L // KT                  # 16 kpos tiles per batch
QB = 512                        # query block (psum bank width)
N_QB = L // QB                  # 4 query blocks per batch


def rne_f32r(a):
    """Round a float32 array to the float32r grid (RNE, drop 12 bits)."""
    a = np.ascontiguousarray(a, dtype=np.float32)
    u = a.view(np.uint32)
    lsb = (u >> 12) & 1
    r = (u.astype(np.uint64) + 0x7FF + lsb) >> 12 << 12
    return r.astype(np.uint32).view(np.float32)


DEBUG = False


def build(dup=1, no_cc=False, stage='full'):
    nc = bacc.Bacc("TRN2", target_bir_lowering=False, debug=False,
                   num_devices=N_CORES)

    # ---- I/O ----
    xt = nc.dram_tensor("xt", [D, BL], F32R, kind="ExternalInput").ap()
    w_sh = nc.dram_tensor("w_sh", [D, 5 * EV], F32R, kind="ExternalInput").ap()
    w_out = nc.dram_tensor("w_out", [D, D], BF16, kind="ExternalInput").ap()
    cos_pk = nc.dram_tensor("cos_pk", [EV, L], F32, kind="ExternalInput").ap()
    sin_pk = nc.dram_tensor("sin_pk", [EV, L], F32, kind="ExternalInput").ap()
    p2t = nc.dram_tensor("p2t", [EV, EV], F32R, kind="ExternalInput").ap()
    ones2 = nc.dram_tensor("ones2", [128, 2], F32R, kind="ExternalInput").ap()
    tril = nc.dram_tensor("tril", [KT, KT], F32, kind="ExternalInput").ap()
    bias8 = nc.dram_tensor("bias8", [128, D // 128], F32,
                           kind="ExternalInput").ap()
    ident_in = nc.dram_tensor("ident_in", [128, 128], F32R,
                              kind="ExternalInput").ap()
    out = nc.dram_tensor("out", [D, SEQ_SH], F32, kind="ExternalOutput").ap()

    dbg = {}
    if DEBUG:
        for nm, shp in [("dbg_q0", [64, BL]), ("dbg_k0", [64, BL]),
                        ("dbg_va0", [128, 130]), ("dbg_attn0", [64, BL])]:
            dbg[nm] = nc.dram_tensor(nm, shp, F32, kind="ExternalOutput").ap()
    nc._dbg = dbg
    nc._no_cc = no_cc
    nc._stage = stage
    with tile.TileContext(nc) as tc:
        for it in range(dup):
            _emit(nc, tc, it, xt, w_sh, w_out, cos_pk, sin_pk, p2t, ones2,
                  tril, bias8, ident_in, out)
    nc.compile()
    return nc


def _emit(nc, tc, it, xt, w_sh, w_out, cos_pk, sin_pk, p2t, ones2, tril,
          bias8, ident_in, out):
    from contextlib import ExitStack
    s = f"_{it}"
    bounce_in = nc.dram_tensor(f"bounce_in{s}", [N_CORES * EV, SEQ_SH], BF16)
    bounce_out = nc.dram_tensor(f"bounce_out{s}", [N_CORES * EV, SEQ_SH], BF16)

    with ExitStack() as ctx:
        # ---- persistent pools (live through attention) ----
        qk_pool = ctx.enter_context(tc.tile_pool(name=f"qk{s}", bufs=1))
        vaug_pool = ctx.enter_context(tc.tile_pool(name=f"vaug{s}", bufs=1))
        att_pool = ctx.enter_context(tc.tile_pool(name=f"att{s}", bufs=1))
        const_pool = ctx.enter_context(tc.tile_pool(name=f"const{s}", bufs=1))

        # split q/k: per head [64, BL] tiles (matmul operands need base
        # partition 0, so each head gets its own tile)
        q_h = [qk_pool.tile([64, BL], F32R, name=f"q_h{h}{s}", tag=f"q_h{h}")
               for h in range(2)]
        k_h = [qk_pool.tile([64, BL], F32R, name=f"k_h{h}{s}", tag=f"k_h{h}")
               for h in range(2)]
        # v_aug: per kpos-tile [128, 2*65]: per head 64 V cols + ones col
        v_aug = [vaug_pool.tile([128, 130], F32R, name=f"vaug{i}{s}",
                                tag=f"vaug{i}")
                 for i in range(2 * N_KT)]
        attn_h = [att_pool.tile([64, BL], BF16, name=f"attn{h}{s}",
                                tag=f"attn{h}")
                  for h in range(2)]
        ones_sb = const_pool.tile([128, 2], F32, name=f"ones_sb{s}", tag="ones")
        nc.vector.memset(ones_sb[:], 1.0)
        tril_sb = const_pool.tile([KT, KT], F32, name=f"tril{s}", tag="tril")
        nc.sync.dma_start(tril_sb[:], tril[:])
        bias_sb = const_pool.tile([128, D // 128], F32, name=f"bias{s}",
                                  tag="bias")
        nc.sync.dma_start(bias_sb[:], bias8[:])

        # ================= Phase A: projections + rope =================
        with ExitStack() as actx:
            apool = actx.enter_context(tc.tile_pool(name=f"pa{s}", bufs=3))
            wpool = actx.enter_context(tc.tile_pool(name=f"pw{s}", bufs=1))
            ppool = actx.enter_context(
                tc.tile_pool(name=f"pap{s}", bufs=2, space="PSUM"))
            tpool = actx.enter_context(
                tc.tile_pool(name=f"pat{s}", bufs=2, space="PSUM"))


            # weights + rope tables + P2 resident
            w_tiles = []
            for c in range(8):
                wt = wpool.tile([128, 5 * EV], F32R, name=f"w{c}{s}", tag=f"w{c}")
                nc.sync.dma_start(wt[:], w_sh[c * 128:(c + 1) * 128, :])
                w_tiles.append(wt)
            cos_sb = wpool.tile([EV, L], F32, name=f"cos{s}", tag="cos")
            nc.sync.dma_start(cos_sb[:], cos_pk[:])
            sin_sb = wpool.tile([EV, L], F32, name=f"sin{s}", tag="sin")
            nc.sync.dma_start(sin_sb[:], sin_pk[:])
            ident = wpool.tile([128, 128], F32R, name=f"ident{s}", tag="ident")
            nc.sync.dma_start(ident[:], ident_in[:])

            for ch in range(N_CH):
                c0 = ch * CHUNK
                xt_big = apool.tile([128, 8 * CHUNK], F32R,
                                    name=f"x_{ch}{s}", tag="xt")
                xt_src = xt.rearrange("(c p) n -> p c n", p=128)
                nc.sync.dma_start(
                    xt_big[:].rearrange("p (c n) -> p c n", c=8),
                    xt_src[:, :, c0:c0 + CHUNK])
                xt_t = [xt_big[:, c * CHUNK:(c + 1) * CHUNK] for c in range(8)]

                bcol = 0 if c0 < L else 1
                lcol = c0 - bcol * L
                # q, k, qrot, krot, v projections back-to-back on PE;
                # rope combines read the psums directly (no copies, no rot
                # matmul: rot weights folded on host as W @ P2.T)
                ps_big = ppool.tile([128, 5 * CHUNK], F32, tag="proj",
                                    name=f"psbig_{ch}{s}")
                ps5 = [ps_big[:, e5 * CHUNK:(e5 + 1) * CHUNK]
                       for e5 in range(5)]
                for e5 in range(5):
                    for c in range(8):
                        nc.tensor.matmul(
                            ps5[e5][:], w_tiles[c][:, e5 * EV:(e5 + 1) * EV],
                            xt_t[c][:], start=(c == 0), stop=(c == 7))
                vraw = apool.tile([128, CHUNK], F32R, tag="vraw",
                                  name=f"vraw_{ch}{s}")
                nc.scalar.copy(vraw[:], ps5[4][:])
                for blk in range(CHUNK // 128):
                    kti = (c0 + blk * 128) // 128  # global kpos tile index
                    pst = tpool.tile([128, 128], F32R, tag="tr",
                                     name=f"pst{blk}_{ch}{s}")
                    nc.tensor.transpose(
                        pst[:], vraw[:, blk * 128:(blk + 1) * 128], ident[:])
                    va = v_aug[kti]
                    nc.scalar.copy(va[:, 0:64], pst[:, 0:64])
                    nc.scalar.copy(va[:, 65:129], pst[:, 64:128])
                    ones_cols = (va[:].rearrange("p (c w) -> p c w", w=65)
                                 [:, :, 64:65])
                    nc.gpsimd.tensor_copy(ones_cols, ones_sb[:])
                for qk, dsts in enumerate([q_h, k_h]):
                    t1 = apool.tile([128, CHUNK], F32, tag=f"t1{qk}",
                                    name=f"t1{qk}_{ch}{s}")
                    nc.vector.tensor_mul(
                        t1[:], ps5[qk][:], cos_sb[:, lcol:lcol + CHUNK])
                    t2 = apool.tile([128, CHUNK], F32, tag=f"t2{qk}",
                                    name=f"t2{qk}_{ch}{s}")
                    nc.vector.tensor_mul(
                        t2[:], ps5[2 + qk][:], sin_sb[:, lcol:lcol + CHUNK])
                    pkc = apool.tile([128, CHUNK], F32R, tag=f"pkc{qk}",
                                     name=f"pkc{qk}_{ch}{s}")
                    nc.gpsimd.tensor_add(pkc[:], t1[:], t2[:])
                    # split heads into base-0 per-head tiles (SB2SB DMA)
                    nc.sync.dma_start(dsts[0][:, c0:c0 + CHUNK], pkc[0:64, :])
                    nc.sync.dma_start(dsts[1][:, c0:c0 + CHUNK],
                                      pkc[64:128, :])

        if getattr(nc, "_stage", "full") == "A":
            nc.sync.dma_start(out[0:64, :], q_h[0][:, 0:SEQ_SH].bitcast(F32))
            return
        # ================= Phase B: attention (+ per-head A2A) ===========
        QB2 = 1024
        N_QB2 = L // QB2
        with ExitStack() as bctx:
            epool = bctx.enter_context(tc.tile_pool(name=f"pe{s}", bufs=6))
            spool = bctx.enter_context(
                tc.tile_pool(name=f"ps{s}", bufs=2, space="PSUM"))
            opool = bctx.enter_context(
                tc.tile_pool(name=f"po{s}", bufs=1, space="PSUM"))
            npool = bctx.enter_context(tc.tile_pool(name=f"pn{s}", bufs=4))

            pending_norms = []
            for h in range(2):
                for b in range(B):
                    boff = b * L
                    pv_ps = [opool.tile([65, QB2], F32, tag=f"pv{qb}",
                                        name=f"pv{b}{h}{qb}{s}")
                             for qb in range(N_QB2)]

                    def emit_scores(ki):
                        kcols = slice(boff + ki * KT, boff + (ki + 1) * KT)
                        work = []
                        for qb in range(ki * KT // QB2, N_QB2):
                            qlo = max(qb * QB2, ki * KT)
                            qhi = (qb + 1) * QB2
                            segs = []
                            p = qlo
                            for edge in (qb * QB2 + 512, qhi):
                                if p < edge:
                                    segs.append((p, edge))
                                    p = edge
                            sc = spool.tile([128, QB2], F32, tag="sc",
                                            name=f"sc{b}{h}{ki}{qb}{s}")
                            for m0, m1 in segs:
                                nc.tensor.matmul(
                                    sc[:, m0 - qb * QB2:m1 - qb * QB2],
                                    k_h[h][:, kcols],
                                    q_h[h][:, boff + m0:boff + m1],
                                    start=True, stop=True)
                            ex = epool.tile([128, QB2], F32R, tag="ex",
                                            name=f"ex{b}{h}{ki}{qb}{s}")
                            nc.scalar.activation(
                                ex[:, 0:qhi - qlo],
                                sc[:, qlo - qb * QB2:QB2],
                                mybir.ActivationFunctionType.Exp)
                            if qlo == ki * KT:
                                nc.vector.tensor_mul(
                                    ex[:, 0:KT], ex[:, 0:KT], tril_sb[:])
                            work.append((qb, qlo, segs, ex))
                        return work

                    def emit_pv(ki, work):
                        last = {qb: qb * (QB2 // KT) + QB2 // KT - 1
                                for qb in range(N_QB2)}
                        for qb, qlo, segs, ex in work:
                            for m0, m1 in segs:
                                nc.tensor.matmul(
                                    pv_ps[qb][:,
                                              m0 - qb * QB2:m1 - qb * QB2],
                                    v_aug[b * N_KT + ki][:,
                                                         h * 65:h * 65 + 65],
                                    ex[:, m0 - qlo:m1 - qlo],
                                    start=(ki == 0), stop=(ki == last[qb]))

                    def emit_norm(qb, pv_ps_=None, boff_=None, h_=None,
                                  b_=None):
                        pv_sb = npool.tile([65, QB2], F32, tag="pv_sb",
                                           name=f"pvsb{b_}{h_}{qb}{s}")
                        nc.vector.tensor_copy(pv_sb[:], pv_ps_[qb][:])
                        sums = npool.tile([1, QB2], F32, tag="sums",
                                          name=f"sums{b_}{h_}{qb}{s}")
                        nc.sync.dma_start(sums[:], pv_sb[64:65, :])
                        recip = npool.tile([1, QB2], F32, tag="recip",
                                           name=f"recip{b_}{h_}{qb}{s}")
                        nc.vector.reciprocal(recip[:], sums[:])
                        bc = npool.tile([64, QB2], F32, tag="bc",
                                        name=f"bc{b_}{h_}{qb}{s}")
                        nc.gpsimd.partition_broadcast(bc[:], recip[:])
                        nc.gpsimd.tensor_mul(
                            attn_h[h_][:,
                                       boff_ + qb * QB2:
                                       boff_ + (qb + 1) * QB2],
                            pv_sb[0:64, :], bc[:])
                        for j in range(QB2 // SEQ_SH):
                            sx = b_ * (L // SEQ_SH) + qb * (QB2 // SEQ_SH) + j
                            c0s = boff_ + qb * QB2 + j * SEQ_SH
                            nc.sync.dma_start(
                                bounce_in[sx * EV + h_ * 64:
                                          sx * EV + (h_ + 1) * 64, :],
                                attn_h[h_][:, c0s:c0s + SEQ_SH])

                    prev = None
                    for ki in range(N_KT):
                        cur = (ki, emit_scores(ki))
                        if prev is not None:
                            emit_pv(*prev)
                        # deferred norms from the previous (b,h) drain here,
                        # hidden behind this iteration's scores/exp
                        if ki in (2, 4) and pending_norms:
                            pending_norms.pop(0)()
                        prev = cur
                    emit_pv(*prev)
                    import functools
                    for qb in range(N_QB2):
                        pending_norms.append(functools.partial(
                            emit_norm, qb, pv_ps_=pv_ps, boff_=boff,
                            h_=h, b_=b))
                # head h done on both batches: drain norms (each norm also
                # stages its bounce pieces, so the A2A can fire immediately)
                while pending_norms:
                    pending_norms.pop(0)()
            # one AllToAll for both heads (each call pays a ~30us ncfw floor)
            if getattr(nc, "_no_cc", False):
                nc.sync.dma_start(bounce_out[:], bounce_in[:])
            else:
                nc.gpsimd.collective_compute(
                    "AllToAll", mybir.AluOpType.bypass,
                    replica_groups=[list(range(N_CORES))],
                    ins=[bounce_in[:].opt()],
                    outs=[bounce_out[:].opt()],
                )

        if getattr(nc, "_stage", "full") == "AB":
            nc.sync.dma_start(out[0:64, :], attn_h[0][:, 0:SEQ_SH].bitcast(F32))
            return
        # ================= Phase C: output projection =================
        with ExitStack() as cctx:
            wopool = cctx.enter_context(tc.tile_pool(name=f"pwo{s}", bufs=1))
            rbpool = cctx.enter_context(tc.tile_pool(name=f"prb{s}", bufs=1))
            fpool = cctx.enter_context(tc.tile_pool(name=f"pf{s}", bufs=4))
            fps = cctx.enter_context(
                tc.tile_pool(name=f"pfp{s}", bufs=1, space="PSUM"))

            wo_tiles = []
            for c in range(8):
                wt = wopool.tile([128, D], BF16, name=f"wo{c}{s}", tag=f"wo{c}")
                nc.sync.dma_start(wt[:], w_out[c * 128:(c + 1) * 128, :])
                wo_tiles.append(wt)
            if nc._dbg and it == 0:
                d = nc._dbg
                nc.sync.dma_start(d["dbg_q0"][:], q_h[0][:].bitcast(F32))
                nc.sync.dma_start(d["dbg_k0"][:], k_h[0][:].bitcast(F32))
                nc.sync.dma_start(d["dbg_va0"][:], v_aug[0][:].bitcast(F32))
                nc.sync.dma_start(d["dbg_attn0"][:], attn_h[0][:].bitcast(F32))

            po = [fps.tile([128, SEQ_SH], F32, tag=f"podd{dd}",
                           name=f"po{dd}{s}") for dd in range(D // 128)]
            rb = []
            for c in range(8):
                t = rbpool.tile([128, SEQ_SH], BF16, name=f"rb{c}{s}",
                                tag=f"rb{c}")
                nc.sync.dma_start(
                    t[:], bounce_out[c * 128:(c + 1) * 128, :])
                rb.append(t)
            for dd in range(D // 128):
                for c in range(8):
                    nc.tensor.matmul(
                        po[dd][:],
                        wo_tiles[c][:, dd * 128:(dd + 1) * 128],
                        rb[c][:], start=(c == 0), stop=(c == 7))
            for dd in range(D // 128):
                fo = fpool.tile([128, SEQ_SH], F32, tag="fo")
                nc.vector.tensor_scalar_add(
                    fo[:], po[dd][:], bias_sb[:, dd:dd + 1])
                nc.sync.dma_start(out[dd * 128:(dd + 1) * 128, :], fo[:])


def _host_prep(x, rope_cos, rope_sin, W_qkv, W_out, b_out):
    """Build per-core input maps."""
    x = np.asarray(x, np.float32)
    rope_cos = np.asarray(rope_cos, np.float32)
    rope_sin = np.asarray(rope_sin, np.float32)
    W_qkv = np.asarray(W_qkv, np.float32)
    W_out = np.asarray(W_out, np.float32)
    b_out = np.asarray(b_out, np.float32)

    xt = rne_f32r(x.reshape(BL, D).T)                      # [D, BL]
    cos_pk = np.ascontiguousarray(
        np.tile(rope_cos[:L].T, (2, 1)))                   # [128, L]
    sin_pk = np.ascontiguousarray(np.tile(rope_sin[:L].T, (2, 1)))
    # rotate-half permutation: (P q)[i] = -q[i+32] (i<32), q[i-32] (i>=32)
    P = np.zeros((HD, HD), np.float32)
    for i in range(32):
        P[i, i + 32] = -1.0
        P[i + 32, i] = 1.0
    P2 = np.zeros((EV, EV), np.float32)
    P2[:HD, :HD] = P
    P2[HD:, HD:] = P
    p2t = rne_f32r(P2.T)
    ident = rne_f32r(np.eye(128, dtype=np.float32))
    ones2 = np.ones((128, 2), np.float32)
    # scoresT layout is [kpos, q]: valid entries have q >= kpos -> upper tri
    tril_m = np.triu(np.ones((KT, KT), np.float32))
    bias8 = np.ascontiguousarray(b_out.reshape(D // 128, 128).T)
    import ml_dtypes
    w_out_r = W_out.astype(ml_dtypes.bfloat16)
    scale = HD ** -0.5

    in_maps = []
    for r in range(N_CORES):
        hs = slice(r * H_PER * HD, (r + 1) * H_PER * HD)
        w_q = W_qkv[:, 0:1024][:, hs] * scale
        w_k = W_qkv[:, 1024:2048][:, hs]
        w_v = W_qkv[:, 2048:3072][:, hs]
        w_sh = rne_f32r(np.concatenate(
            [w_q, w_k, w_q @ P2.T, w_k @ P2.T, w_v], axis=1))
        in_maps.append({
            "xt": xt, "w_sh": w_sh, "w_out": w_out_r,
            "cos_pk": cos_pk, "sin_pk": sin_pk, "p2t": p2t,
            "ident_in": ident, "ones2": ones2, "tril": tril_m,
            "bias8": bias8,
        })
    return in_maps


_NC_CACHE = {}


def kernel(x, rope_cos, rope_sin, W_qkv, W_out, b_out):
    if "nc" not in _NC_CACHE:
        _NC_CACHE["nc"] = build()
    nc = _NC_CACHE["nc"]
    in_maps = _host_prep(x, rope_cos, rope_sin, W_qkv, W_out, b_out)
    res = run_bass_kernel_spmd(nc, in_maps, core_ids=list(range(N_CORES)))
    outp = np.empty((BL, D), np.float32)
    for r in range(N_CORES):
        outp[r * SEQ_SH:(r + 1) * SEQ_SH, :] = res.results[r]["out"].T
    return outp.reshape(B, L, D)



# revision 19
# speedup vs baseline: 3.9092x; 3.9092x over previous
"""Distributed TRN2 Bass kernel for causal multi-head attention
(B=2, L=2048, D=1024, H=16, HD=64) on 8 NeuronCores.

Sharding: tensor-parallel over heads — 2 heads per core, full sequence on
every core. Two 8-core AllToAlls (one per local head) re-shard the per-head
attention outputs by sequence block; core r computes output rows
[r*512, (r+1)*512) of the output projection. The host concatenates the 8
slices. The h0 AllToAll overlaps the h1 attention compute; the h0 half of
the output projection overlaps the h1 AllToAll.

Design notes:
- QKV projection computes q, k, v (3x128 cols). Rotate-half is one extra
  512-col PE matmul per chunk against the block-diagonal permutation P2
  (q,k copied psum->SBUF by the otherwise-idle ACT engine). The rope
  combine is a pure per-chunk dataflow (2 DVE muls + 1 Pool add).
- Everything DMA-fed is bf16 (x, weights, rope tables, q/k/v, exp scores,
  bounce, out). PE runs 1 cycle/row on bf16; FWL keeps LDWEIGHTS off the
  critical path. PSUM accumulation stays fp32.
- PV is computed transposed: stationary = exp-scores tile [kpos, q-block],
  moving = v_aug [kpos, 65] -> psum [q, 64+den]. The softmax denominator
  lands per-partition, so normalization is reciprocal[128,1] +
  tensor_scalar mul — no partition broadcasts. PSUM accumulation groups
  are strictly sequential per bank (start=True clears has_written
  bank-wide, so interleaved groups corrupt each other).
- V and attention-output transposes use the DMA XBAR transpose; its out AP
  must be [p, nblk, rows] with the last dim contiguous — strided variants
  silently corrupt.
- DMA instruction count is kept small: each dma_start costs ~565ns of SP
  sequencer + ~625ns HWDGE generation.
"""
import numpy as np

import concourse.bass as bass
import concourse.tile as tile
from concourse import bacc, mybir
from concourse.bass_utils import run_bass_kernel_spmd

# problem shape (hardcoded per harness contract)
B, L, D = 2, 2048, 1024
H, HD = 16, 64
BL = B * L                      # 4096
N_CORES = 8
H_PER = H // N_CORES            # 2 heads per core
EV = H_PER * HD                 # 128: packed per-core head dim
SEQ_SH = BL // N_CORES          # 512: output rows per core after A2A

F32 = mybir.dt.float32
BF16 = mybir.dt.bfloat16
CHUNK = 256                     # moving-dim chunk for projection
N_CH = BL // CHUNK              # 16 (8 per batch)
KT = 128                        # kpos tile
N_KT = L // KT                  # 16 kpos tiles per batch
SEG = 1024                      # score/exp segment width (2 psum banks)
N_SEG = L // SEG                # 2 per batch
N_QB = L // KT                  # 16 qblocks (128 wide) per batch


def build(dup=1, no_cc=False, stage='full'):
    nc = bacc.Bacc("TRN2", target_bir_lowering=False, debug=False,
                   num_devices=N_CORES)

    xt = nc.dram_tensor("xt", [D, BL], BF16, kind="ExternalInput").ap()
    w3 = nc.dram_tensor("w3", [D, 3 * EV], BF16, kind="ExternalInput").ap()
    p2t = nc.dram_tensor("p2t", [EV, EV], BF16, kind="ExternalInput").ap()
    wo = nc.dram_tensor("wo", [D, D], BF16, kind="ExternalInput").ap()
    cos_pk = nc.dram_tensor("cos_pk", [EV, L], BF16, kind="ExternalInput").ap()
    sin_pm = nc.dram_tensor("sin_pm", [EV, L], BF16, kind="ExternalInput").ap()
    tril = nc.dram_tensor("tril", [KT, KT], BF16, kind="ExternalInput").ap()
    bias8 = nc.dram_tensor("bias8", [128, D // 128], F32,
                           kind="ExternalInput").ap()
    out = nc.dram_tensor("out", [D, SEQ_SH], BF16, kind="ExternalOutput").ap()

    nc._no_cc = no_cc
    nc._stage = stage
    with tile.TileContext(nc) as tc:
        for it in range(dup):
            _emit(nc, tc, it, xt, w3, p2t, wo, cos_pk, sin_pm, tril, bias8,
                  out)
    nc.compile()
    return nc


def _emit(nc, tc, it, xt, w3, p2t, wo, cos_pk, sin_pm, tril, bias8, out):
    from contextlib import ExitStack
    s = f"_{it}"
    # bounce: rows = sx*128 + h*64 + d (d = head-local ev dim)
    bnc_in = nc.dram_tensor(f"bnc_in{s}", [N_CORES * EV, SEQ_SH], BF16)
    bnc_out = nc.dram_tensor(f"bnc_out{s}", [N_CORES * EV, SEQ_SH], BF16)

    def x_load(pool, b, ch):
        c0g = b * L + ch * CHUNK
        xt_big = pool.tile([128, 8 * CHUNK], BF16,
                           name=f"x{b}{ch}{s}", tag="xt")
        nc.sync.dma_start(
            xt_big[:].rearrange("p (c n) -> p c n", c=8),
            xt.rearrange("(c p) n -> p c n", p=128)[:, :, c0g:c0g + CHUNK])
        return xt_big

    with ExitStack() as ctx:
        # ---- persistent pools ----
        cpool = ctx.enter_context(tc.tile_pool(name=f"const{s}", bufs=1))
        qkpool = ctx.enter_context(tc.tile_pool(name=f"qk{s}", bufs=1))
        vpool = ctx.enter_context(tc.tile_pool(name=f"v{s}", bufs=1))

        w3_all = cpool.tile([128, 8 * 3 * EV], BF16, name=f"w3a{s}", tag="w3a")
        nc.sync.dma_start(
            w3_all[:].rearrange("p (c w) -> p c w", c=8),
            w3.rearrange("(c p) w -> p c w", p=128))
        p2_sb = cpool.tile([EV, EV], BF16, name=f"p2{s}", tag="p2")
        nc.sync.dma_start(p2_sb[:], p2t[:])
        wo_all = cpool.tile([128, 8 * D], BF16, name=f"woa{s}", tag="woa")
        cos_sb = cpool.tile([EV, L], BF16, name=f"cos{s}", tag="cos")
        sin_sb = cpool.tile([EV, L], BF16, name=f"sin{s}", tag="sin")
        tril_sb = cpool.tile([KT, KT], BF16, name=f"tril{s}", tag="tril")
        bias_sb = cpool.tile([128, D // 128], F32, name=f"bias{s}", tag="bias")

        # per-head q/k over full sequence
        q_h = [qkpool.tile([64, BL], BF16, name=f"q{h}{s}", tag=f"q{h}")
               for h in range(2)]
        k_h = [qkpool.tile([64, BL], BF16, name=f"k{h}{s}", tag=f"k{h}")
               for h in range(2)]
        # v_aug per batch: 16 kt blocks of (64 v | 1)(64 v | 1), stride 130
        va = [vpool.tile([128, N_KT * 130], BF16, name=f"va{b}{s}",
                         tag=f"va{b}") for b in range(B)]

        # ================= Phase A: projections + rope =================
        with ExitStack() as actx:
            apool = actx.enter_context(tc.tile_pool(name=f"pa{s}", bufs=5))
            rpool = actx.enter_context(tc.tile_pool(name=f"pr{s}", bufs=2))
            ppool = actx.enter_context(
                tc.tile_pool(name=f"pap{s}", bufs=2, space="PSUM"))

            # prime the DMA queue: first x chunks before table loads
            xq = {(0, 0): x_load(apool, 0, 0), (0, 1): x_load(apool, 0, 1)}
            nc.sync.dma_start(cos_sb[:], cos_pk[:])
            nc.sync.dma_start(sin_sb[:], sin_pm[:])
            nc.sync.dma_start(tril_sb[:], tril[:])
            nc.sync.dma_start(bias_sb[:], bias8[:])
            for b in range(B):
                nc.vector.memset(
                    va[b][:].rearrange("p (m w) -> p m w", w=65)[:, :, 64:65],
                    1.0)

            for b in range(B):
                pk = [rpool.tile([128, L], BF16, name=f"pk{qk}{b}{s}",
                                 tag=f"pk{qk}") for qk in range(2)]
                v_sb = rpool.tile([128, L], BF16, name=f"vsb{b}{s}", tag="vsb")
                for ch in range(N_CH // B):
                    lc = ch * CHUNK               # batch-local column
                    xt_big = xq.pop((b, ch), None)
                    if xt_big is None:
                        xt_big = x_load(apool, b, ch)
                    ps3 = ppool.tile([128, 3 * CHUNK], F32, tag="ps3",
                                     name=f"ps3{b}{ch}{s}")
                    for e3 in range(3):
                        for c in range(8):
                            nc.tensor.matmul(
                                ps3[:, e3 * CHUNK:(e3 + 1) * CHUNK],
                                w3_all[:, c * 384 + e3 * 128:
                                       c * 384 + (e3 + 1) * 128],
                                xt_big[:, c * CHUNK:(c + 1) * CHUNK],
                                start=(c == 0), stop=(c == 7))
                    nc.vector.tensor_copy(v_sb[:, lc:lc + CHUNK],
                                          ps3[:, 2 * CHUNK:3 * CHUNK])
                    qsb = apool.tile([128, 2 * CHUNK], BF16, tag="qsb",
                                     name=f"qsb{b}{ch}{s}")
                    nc.scalar.copy(qsb[:], ps3[:, 0:2 * CHUNK])
                    rot = ppool.tile([128, 2 * CHUNK], F32, tag="rot",
                                     name=f"rot{b}{ch}{s}")
                    nc.tensor.matmul(rot[:], p2_sb[:], qsb[:],
                                     start=True, stop=True)
                    for qk in range(2):
                        t1 = apool.tile([128, CHUNK], F32, tag=f"t1{qk}",
                                        name=f"t1{qk}{b}{ch}{s}")
                        nc.vector.tensor_mul(
                            t1[:], ps3[:, qk * CHUNK:(qk + 1) * CHUNK],
                            cos_sb[:, lc:lc + CHUNK])
                        t2 = apool.tile([128, CHUNK], F32, tag=f"t2{qk}",
                                        name=f"t2{qk}{b}{ch}{s}")
                        nc.vector.tensor_mul(
                            t2[:], rot[:, qk * CHUNK:(qk + 1) * CHUNK],
                            sin_sb[:, lc:lc + CHUNK])
                        nc.gpsimd.tensor_add(pk[qk][:, lc:lc + CHUNK],
                                             t1[:], t2[:])
                # prefetch next batch's first x chunks ahead of tail DMAs
                if b + 1 < B:
                    xq[(b + 1, 0)] = x_load(apool, b + 1, 0)
                    xq[(b + 1, 1)] = x_load(apool, b + 1, 1)
                # batch b done: split heads, transpose v.
                for qk, dsts in enumerate([q_h, k_h]):
                    nc.sync.dma_start(dsts[0][:, b * L:(b + 1) * L],
                                      pk[qk][0:64, :])
                    nc.sync.dma_start(dsts[1][:, b * L:(b + 1) * L],
                                      pk[qk][64:128, :])
                # XBAR transpose needs a contiguous [p, j, rows] out AP;
                # redistribute into the 65-stride va layout with a plain
                # strided SB2SB copy.
                va_j = va[b][:].rearrange("p (j w) -> p j w", w=130)
                for hf in range(2):
                    vh = rpool.tile([128, N_KT * 64], BF16, tag=f"vh{hf}",
                                    name=f"vh{hf}{b}{s}")
                    nc.sync.dma_start(
                        vh[:].rearrange("p (j c) -> p j c", c=64),
                        v_sb[hf * 64:(hf + 1) * 64, :], transpose=True)
                    nc.sync.dma_start(
                        va_j[:, :, hf * 65:hf * 65 + 64],
                        vh[:].rearrange("p (j c) -> p j c", c=64))

        # phase-C weights: load during attention (DMA device is idle then)
        nc.sync.dma_start(
            wo_all[:].rearrange("p (g w) -> p g w", g=8),
            wo.rearrange("(g p) w -> p g w", p=128))

        if getattr(nc, "_stage", "full") == "A":
            nc.sync.dma_start(out[0:64, :], q_h[0][:, 0:SEQ_SH])
            return

        # ================= Phase B: attention =================
        with ExitStack() as bctx:
            epool = bctx.enter_context(tc.tile_pool(name=f"pe{s}", bufs=17))
            npool = bctx.enter_context(tc.tile_pool(name=f"pn{s}", bufs=2))
            o2pool = bctx.enter_context(tc.tile_pool(name=f"po2{s}", bufs=2))
            spool = bctx.enter_context(
                tc.tile_pool(name=f"ps{s}", bufs=2, space="PSUM"))
            pvpool = bctx.enter_context(
                tc.tile_pool(name=f"ppv{s}", bufs=1, space="PSUM"))

            def stage_head(b, h, attn2h):
                # transpose [q, j*64+d] -> stg [jpar*64+d, cb, q] and stage
                # both parities into the per-head bounce (rows sx*64 + d)
                stg = o2pool.tile([128, L // 2], BF16, tag=f"stg{h}",
                                  name=f"stg{b}{h}{s}")
                nc.sync.dma_start(
                    stg[:].rearrange("p (cb r) -> p cb r", r=128),
                    attn2h[:], transpose=True)
                for par in range(2):
                    for cb2 in range(2):
                        dst = bnc_in.rearrange(
                            "(sx e) (cb2 pr r) -> e sx cb2 pr r",
                            e=128, cb2=2, pr=2)[
                            h * 64:(h + 1) * 64, 4 * b:4 * b + 4, cb2, par]
                        src = stg[par * 64:(par + 1) * 64, :].rearrange(
                            "d (sx cb2 r) -> d sx cb2 r",
                            sx=4, cb2=2)[:, :, cb2]
                        nc.sync.dma_start(dst, src)

            attn2h = None
            for b in range(B):
                boff = b * L
                for h in range(2):
                    attn2h = o2pool.tile([128, N_QB * 64], BF16,
                                         name=f"at2{b}{h}{s}", tag=f"at2{h}")
                    # pvT psum: qblocks packed at stride 65, split 7/7/2 so
                    # no matmul output crosses a psum bank.
                    pv_t = [pvpool.tile([128, n * 65], F32, tag=f"pv{i}",
                                        name=f"pv{b}{h}{i}{s}")
                            for i, n in enumerate((7, 7, 2))]

                    def pv_ap(j):
                        t, jj = (0, j) if j < 7 else \
                                (1, j - 7) if j < 14 else (2, j - 14)
                        return pv_t[t][:, jj * 65:(jj + 1) * 65]

                    den_sb = npool.tile([128, N_QB], F32, tag="den",
                                        name=f"den{b}{h}{s}")
                    ex_t = {}          # (ki, sg) -> (tile, qlo)

                    def emit_scores(ki):
                        kcols = k_h[h][:, boff + ki * KT:
                                       boff + (ki + 1) * KT]
                        for sg in range(ki // 8, N_SEG):
                            qlo = max(sg * SEG, ki * KT)
                            qhi = (sg + 1) * SEG
                            sc = spool.tile([128, SEG], F32, tag="sc",
                                            name=f"sc{b}{h}{ki}{sg}{s}")
                            for half in range(2):
                                m0 = max(qlo, sg * SEG + half * 512)
                                m1 = sg * SEG + (half + 1) * 512
                                if m0 < m1:
                                    nc.tensor.matmul(
                                        sc[:, m0 - sg * SEG:m1 - sg * SEG],
                                        kcols,
                                        q_h[h][:, boff + m0:boff + m1],
                                        start=True, stop=True)
                            ex = epool.tile([128, SEG], BF16, tag=f"ex{sg}",
                                            name=f"ex{b}{h}{ki}{sg}{s}")
                            nc.scalar.activation(
                                ex[:, 0:qhi - qlo],
                                sc[:, qlo - sg * SEG:SEG],
                                mybir.ActivationFunctionType.Exp)
                            if qlo == ki * KT:
                                nc.gpsimd.tensor_mul(
                                    ex[:, 0:KT], ex[:, 0:KT], tril_sb[:])
                            ex_t[(ki, sg)] = (ex, qlo)

                    def emit_pv(j):
                        # one sequential psum accumulation group per qblock
                        sg = j // 8
                        for kk in range(j + 1):
                            ex, qlo = ex_t[(kk, sg)]
                            nc.tensor.matmul(
                                pv_ap(j),
                                ex[:, j * KT - qlo:(j + 1) * KT - qlo],
                                va[b][:, kk * 130 + h * 65:
                                      kk * 130 + (h + 1) * 65],
                                start=(kk == 0), stop=(kk == j))
                        if j in (6, 13, 15):
                            t0 = 0 if j == 6 else (7 if j == 13 else 14)
                            n = j - t0 + 1
                            ti = 0 if j == 6 else (1 if j == 13 else 2)
                            nc.vector.reciprocal(
                                den_sb[:, t0:t0 + n],
                                pv_t[ti][:].rearrange(
                                    "p (j w) -> p j w", w=65)[:, :, 64])
                            for jj in range(t0, j + 1):
                                nc.vector.tensor_scalar_mul(
                                    attn2h[:, jj * 64:(jj + 1) * 64],
                                    pv_ap(jj)[:, 0:64],
                                    den_sb[:, jj:jj + 1])

                    for ki in range(N_KT):
                        emit_scores(ki)
                        if ki >= 1:
                            emit_pv(ki - 1)
                    emit_pv(N_KT - 1)
                    stage_head(b, h, attn2h)
            if getattr(nc, "_no_cc", False):
                nc.sync.dma_start(bnc_out[:], bnc_in[:])
            elif getattr(nc, "_stage", "full") == "full":
                nc.gpsimd.collective_compute(
                    "AllToAll", mybir.AluOpType.bypass,
                    replica_groups=[list(range(N_CORES))],
                    ins=[bnc_in[:].opt()],
                    outs=[bnc_out[:].opt()],
                )

            if getattr(nc, "_stage", "full") in ("AB", "ABraw"):
                nc.sync.dma_start(out[0:128, :], attn2h[:, 0:SEQ_SH])
                return

        # ================= Phase C: output projection =================
        with ExitStack() as cctx:
            fpool = cctx.enter_context(tc.tile_pool(name=f"pf{s}", bufs=1))
            fps = cctx.enter_context(
                tc.tile_pool(name=f"pfp{s}", bufs=1, space="PSUM"))

            po = [fps.tile([128, SEQ_SH], F32, tag=f"po{dd}",
                           name=f"po{dd}{s}") for dd in range(D // 128)]
            for half in range(2):
                rbh = fpool.tile([128, 4 * SEQ_SH], BF16,
                                 name=f"rb{half}{s}", tag=f"rb{half}")
                for t in range(4):
                    g = half * 4 + t
                    nc.sync.dma_start(
                        rbh[:, t * SEQ_SH:(t + 1) * SEQ_SH],
                        bnc_out[g * 128:(g + 1) * 128, :])
                for dd in range(D // 128):
                    for t in range(4):
                        g = half * 4 + t
                        nc.tensor.matmul(
                            po[dd][:],
                            wo_all[:, g * D + dd * 128:
                                   g * D + (dd + 1) * 128],
                            rbh[:, t * SEQ_SH:(t + 1) * SEQ_SH],
                            start=(g == 0), stop=(g == 7))
            fo = fpool.tile([128, D // 128 * SEQ_SH], BF16, name=f"fo{s}",
                            tag="fo")
            for dd in range(D // 128):
                nc.vector.tensor_scalar_add(
                    fo[:, dd * SEQ_SH:(dd + 1) * SEQ_SH], po[dd][:],
                    bias_sb[:, dd:dd + 1])
            nc.sync.dma_start(
                out.rearrange("(dd p) c -> p dd c", p=128),
                fo[:].rearrange("p (dd c) -> p dd c", dd=8))


def _host_prep(x, rope_cos, rope_sin, W_qkv, W_out, b_out):
    """Build per-core input maps."""
    import ml_dtypes
    bf = ml_dtypes.bfloat16
    x = np.asarray(x, np.float32)
    rope_cos = np.asarray(rope_cos, np.float32)
    rope_sin = np.asarray(rope_sin, np.float32)
    W_qkv = np.asarray(W_qkv, np.float32)
    W_out = np.asarray(W_out, np.float32)
    b_out = np.asarray(b_out, np.float32)

    xt = np.ascontiguousarray(x.reshape(BL, D).T).astype(bf)     # [D, BL]
    cos_pk = np.ascontiguousarray(
        np.tile(rope_cos[:L].T, (2, 1))).astype(bf)              # [128, L]
    sin_pm = np.ascontiguousarray(
        np.tile(rope_sin[:L].T, (2, 1))).astype(bf)              # [128, L]
    # scoresT layout is [kpos, q]: valid entries have q >= kpos -> upper tri
    tril_m = np.triu(np.ones((KT, KT), np.float32)).astype(bf)
    bias8 = np.ascontiguousarray(b_out.reshape(D // 128, 128).T)
    scale = HD ** -0.5

    # rotate-half permutation: (P q)[i] = -q[i+32] (i<32), q[i-32] (i>=32)
    P = np.zeros((HD, HD), np.float32)
    for i in range(32):
        P[i, i + 32] = -1.0
        P[i + 32, i] = 1.0
    P2 = np.zeros((EV, EV), np.float32)
    P2[:HD, :HD] = P
    P2[HD:, HD:] = P
    p2t_b = np.ascontiguousarray(P2.T).astype(bf)

    wo_h = np.ascontiguousarray(W_out).astype(bf)

    in_maps = []
    for r in range(N_CORES):
        hs = slice(r * H_PER * HD, (r + 1) * H_PER * HD)
        w_q = W_qkv[:, 0:1024][:, hs] * scale
        w_k = W_qkv[:, 1024:2048][:, hs]
        w_v = W_qkv[:, 2048:3072][:, hs]
        w3 = np.ascontiguousarray(np.concatenate(
            [w_q, w_k, w_v], axis=1)).astype(bf)
        in_maps.append({
            "xt": xt, "w3": w3, "p2t": p2t_b, "wo": wo_h,
            "cos_pk": cos_pk, "sin_pm": sin_pm,
            "tril": tril_m, "bias8": bias8,
        })
    return in_maps


_NC_CACHE = {}


def kernel(x, rope_cos, rope_sin, W_qkv, W_out, b_out):
    if "nc" not in _NC_CACHE:
        _NC_CACHE["nc"] = build()
    nc = _NC_CACHE["nc"]
    in_maps = _host_prep(x, rope_cos, rope_sin, W_qkv, W_out, b_out)
    res = run_bass_kernel_spmd(nc, in_maps, core_ids=list(range(N_CORES)))
    outp = np.empty((BL, D), np.float32)
    for r in range(N_CORES):
        outp[r * SEQ_SH:(r + 1) * SEQ_SH, :] = \
            res.results[r]["out"].astype(np.float32).T
    return outp.reshape(B, L, D)


# revision 20
# speedup vs baseline: 4.4303x; 1.1333x over previous
"""Distributed TRN2 Bass kernel for causal multi-head attention
(B=2, L=2048, D=1024, H=16, HD=64) on 8 NeuronCores.

Sharding: tensor-parallel over heads — 2 heads per core, full sequence on
every core. Two 8-core AllToAlls (one per local head) re-shard the per-head
attention outputs by sequence block; core r computes output rows
[r*512, (r+1)*512) of the output projection. The host concatenates the 8
slices. The h0 AllToAll overlaps the h1 attention compute; the h0 half of
the output projection overlaps the h1 AllToAll.

Design notes:
- QKV projection computes q, k, v (3x128 cols). Rotate-half is one extra
  512-col PE matmul per chunk against the block-diagonal permutation P2
  (q,k copied psum->SBUF by the otherwise-idle ACT engine). The rope
  combine is a pure per-chunk dataflow (2 DVE muls + 1 Pool add).
- Everything DMA-fed is bf16 (x, weights, rope tables, q/k/v, exp scores,
  bounce, out). PE runs 1 cycle/row on bf16; FWL keeps LDWEIGHTS off the
  critical path. PSUM accumulation stays fp32.
- PV is computed transposed: stationary = exp-scores tile [kpos, q-block],
  moving = v_aug [kpos, 65] -> psum [q, 64+den]. The softmax denominator
  lands per-partition, so normalization is reciprocal[128,1] +
  tensor_scalar mul — no partition broadcasts. PSUM accumulation groups
  are strictly sequential per bank (start=True clears has_written
  bank-wide, so interleaved groups corrupt each other).
- V and attention-output transposes use the DMA XBAR transpose; its out AP
  must be [p, nblk, rows] with the last dim contiguous — strided variants
  silently corrupt.
- DMA instruction count is kept small: each dma_start costs ~565ns of SP
  sequencer + ~625ns HWDGE generation.
"""
import numpy as np

import concourse.bass as bass
import concourse.tile as tile
from concourse import bacc, mybir
from concourse.bass_utils import run_bass_kernel_spmd

# problem shape (hardcoded per harness contract)
B, L, D = 2, 2048, 1024
H, HD = 16, 64
BL = B * L                      # 4096
N_CORES = 8
H_PER = H // N_CORES            # 2 heads per core
EV = H_PER * HD                 # 128: packed per-core head dim
SEQ_SH = BL // N_CORES          # 512: output rows per core after A2A

F32 = mybir.dt.float32
BF16 = mybir.dt.bfloat16
CHUNK = 512                     # moving-dim chunk for projection
N_CH = BL // CHUNK              # 16 (8 per batch)
KT = 128                        # kpos tile
N_KT = L // KT                  # 16 kpos tiles per batch
SEG = 1024                      # score/exp segment width (2 psum banks)
N_SEG = L // SEG                # 2 per batch
N_QB = L // KT                  # 16 qblocks (128 wide) per batch


def build(dup=1, no_cc=False, stage='full'):
    nc = bacc.Bacc("TRN2", target_bir_lowering=False, debug=False,
                   num_devices=N_CORES)

    xt = nc.dram_tensor("xt", [D, BL], BF16, kind="ExternalInput").ap()
    w3 = nc.dram_tensor("w3", [D, 3 * EV], BF16, kind="ExternalInput").ap()
    p2t = nc.dram_tensor("p2t", [EV, EV], BF16, kind="ExternalInput").ap()
    wo = nc.dram_tensor("wo", [D, D], BF16, kind="ExternalInput").ap()
    cos_pk = nc.dram_tensor("cos_pk", [EV, L], BF16, kind="ExternalInput").ap()
    sin_pm = nc.dram_tensor("sin_pm", [EV, L], BF16, kind="ExternalInput").ap()
    tril = nc.dram_tensor("tril", [KT, KT], BF16, kind="ExternalInput").ap()
    bias8 = nc.dram_tensor("bias8", [128, D // 128], F32,
                           kind="ExternalInput").ap()
    out = nc.dram_tensor("out", [D, SEQ_SH], BF16, kind="ExternalOutput").ap()

    nc._no_cc = no_cc
    nc._stage = stage
    with tile.TileContext(nc) as tc:
        for it in range(dup):
            _emit(nc, tc, it, xt, w3, p2t, wo, cos_pk, sin_pm, tril, bias8,
                  out)
    nc.compile()
    return nc


def _emit(nc, tc, it, xt, w3, p2t, wo, cos_pk, sin_pm, tril, bias8, out):
    from contextlib import ExitStack
    s = f"_{it}"
    # bounce: rows = sx*128 + h*64 + d (d = head-local ev dim)
    bnc_in = nc.dram_tensor(f"bnc_in{s}", [N_CORES * EV, SEQ_SH], BF16)
    bnc_out = nc.dram_tensor(f"bnc_out{s}", [N_CORES * EV, SEQ_SH], BF16)

    def x_load(pool, b, ch):
        c0g = b * L + ch * CHUNK
        xt_big = pool.tile([128, 8 * CHUNK], BF16,
                           name=f"x{b}{ch}{s}", tag="xt")
        nc.sync.dma_start(
            xt_big[:].rearrange("p (c n) -> p c n", c=8),
            xt.rearrange("(c p) n -> p c n", p=128)[:, :, c0g:c0g + CHUNK])
        return xt_big

    with ExitStack() as ctx:
        # ---- persistent pools ----
        cpool = ctx.enter_context(tc.tile_pool(name=f"const{s}", bufs=1))
        qkpool = ctx.enter_context(tc.tile_pool(name=f"qk{s}", bufs=1))
        vpool = ctx.enter_context(tc.tile_pool(name=f"v{s}", bufs=1))

        w3_all = cpool.tile([128, 8 * 3 * EV], BF16, name=f"w3a{s}", tag="w3a")
        nc.sync.dma_start(
            w3_all[:].rearrange("p (c w) -> p c w", c=8),
            w3.rearrange("(c p) w -> p c w", p=128))
        p2_sb = cpool.tile([EV, EV], BF16, name=f"p2{s}", tag="p2")
        nc.sync.dma_start(p2_sb[:], p2t[:])
        wo_all = cpool.tile([128, 8 * D], BF16, name=f"woa{s}", tag="woa")
        cos_sb = cpool.tile([EV, L], BF16, name=f"cos{s}", tag="cos")
        sin_sb = cpool.tile([EV, L], BF16, name=f"sin{s}", tag="sin")
        tril_sb = cpool.tile([KT, KT], BF16, name=f"tril{s}", tag="tril")
        bias_sb = cpool.tile([128, D // 128], F32, name=f"bias{s}", tag="bias")

        # per-head q/k over full sequence
        q_h = [qkpool.tile([64, BL], BF16, name=f"q{h}{s}", tag=f"q{h}")
               for h in range(2)]
        k_h = [qkpool.tile([64, BL], BF16, name=f"k{h}{s}", tag=f"k{h}")
               for h in range(2)]
        # v_aug per batch: 16 kt blocks of (64 v | 1)(64 v | 1), stride 130
        va = [vpool.tile([128, N_KT * 130], BF16, name=f"va{b}{s}",
                         tag=f"va{b}") for b in range(B)]

        # ================= Phase A: projections + rope =================
        with ExitStack() as actx:
            apool = actx.enter_context(tc.tile_pool(name=f"pa{s}", bufs=5))
            rpool = actx.enter_context(tc.tile_pool(name=f"pr{s}", bufs=2))
            ppool = actx.enter_context(
                tc.tile_pool(name=f"pap{s}", bufs=2, space="PSUM"))
            rotpool = actx.enter_context(
                tc.tile_pool(name=f"rpp{s}", bufs=1, space="PSUM"))

            # prime the DMA queue: first x chunks before table loads
            xq = {(0, 0): x_load(apool, 0, 0), (0, 1): x_load(apool, 0, 1)}
            nc.sync.dma_start(cos_sb[:], cos_pk[:])
            nc.sync.dma_start(sin_sb[:], sin_pm[:])
            nc.sync.dma_start(tril_sb[:], tril[:])
            nc.sync.dma_start(bias_sb[:], bias8[:])
            for b in range(B):
                nc.vector.memset(
                    va[b][:].rearrange("p (m w) -> p m w", w=65)[:, :, 64:65],
                    1.0)

            for b in range(B):
                pk = [rpool.tile([128, L], BF16, name=f"pk{qk}{b}{s}",
                                 tag=f"pk{qk}") for qk in range(2)]
                v_sb = rpool.tile([128, L], BF16, name=f"vsb{b}{s}", tag="vsb")
                for ch in range(N_CH // B):
                    lc = ch * CHUNK               # batch-local column
                    xt_big = xq.pop((b, ch), None)
                    if xt_big is None:
                        xt_big = x_load(apool, b, ch)
                    ps3 = ppool.tile([128, 3 * CHUNK], F32, tag="ps3",
                                     name=f"ps3{b}{ch}{s}")
                    for e3 in range(3):
                        for c in range(8):
                            nc.tensor.matmul(
                                ps3[:, e3 * CHUNK:(e3 + 1) * CHUNK],
                                w3_all[:, c * 384 + e3 * 128:
                                       c * 384 + (e3 + 1) * 128],
                                xt_big[:, c * CHUNK:(c + 1) * CHUNK],
                                start=(c == 0), stop=(c == 7))
                    nc.vector.tensor_copy(v_sb[:, lc:lc + CHUNK],
                                          ps3[:, 2 * CHUNK:3 * CHUNK])
                    qsb = apool.tile([128, 2 * CHUNK], BF16, tag="qsb",
                                     name=f"qsb{b}{ch}{s}")
                    nc.scalar.copy(qsb[:], ps3[:, 0:2 * CHUNK])
                    rot = rotpool.tile([128, 2 * CHUNK], F32, tag="rot",
                                       name=f"rot{b}{ch}{s}")
                    for rh in range(2 * CHUNK // 512):
                        nc.tensor.matmul(rot[:, rh * 512:(rh + 1) * 512],
                                         p2_sb[:],
                                         qsb[:, rh * 512:(rh + 1) * 512],
                                         start=True, stop=True)
                    for qk in range(2):
                        t1 = apool.tile([128, CHUNK], F32, tag=f"t1{qk}",
                                        name=f"t1{qk}{b}{ch}{s}")
                        nc.vector.tensor_mul(
                            t1[:], ps3[:, qk * CHUNK:(qk + 1) * CHUNK],
                            cos_sb[:, lc:lc + CHUNK])
                        t2 = apool.tile([128, CHUNK], F32, tag=f"t2{qk}",
                                        name=f"t2{qk}{b}{ch}{s}")
                        nc.vector.tensor_mul(
                            t2[:], rot[:, qk * CHUNK:(qk + 1) * CHUNK],
                            sin_sb[:, lc:lc + CHUNK])
                        nc.gpsimd.tensor_add(pk[qk][:, lc:lc + CHUNK],
                                             t1[:], t2[:])
                # prefetch next batch's first x chunks ahead of tail DMAs
                if b + 1 < B:
                    xq[(b + 1, 0)] = x_load(apool, b + 1, 0)
                    xq[(b + 1, 1)] = x_load(apool, b + 1, 1)
                # batch b done: split heads, transpose v.
                for qk, dsts in enumerate([q_h, k_h]):
                    nc.sync.dma_start(dsts[0][:, b * L:(b + 1) * L],
                                      pk[qk][0:64, :])
                    nc.sync.dma_start(dsts[1][:, b * L:(b + 1) * L],
                                      pk[qk][64:128, :])
                # XBAR transpose needs a contiguous [p, j, rows] out AP;
                # redistribute into the 65-stride va layout with a plain
                # strided SB2SB copy.
                va_j = va[b][:].rearrange("p (j w) -> p j w", w=130)
                for hf in range(2):
                    vh = rpool.tile([128, N_KT * 64], BF16, tag=f"vh{hf}",
                                    name=f"vh{hf}{b}{s}")
                    nc.sync.dma_start(
                        vh[:].rearrange("p (j c) -> p j c", c=64),
                        v_sb[hf * 64:(hf + 1) * 64, :], transpose=True)
                    nc.sync.dma_start(
                        va_j[:, :, hf * 65:hf * 65 + 64],
                        vh[:].rearrange("p (j c) -> p j c", c=64))

        # phase-C weights: load during attention (DMA device is idle then)
        nc.sync.dma_start(
            wo_all[:].rearrange("p (g w) -> p g w", g=8),
            wo.rearrange("(g p) w -> p g w", p=128))

        if getattr(nc, "_stage", "full") == "A":
            nc.sync.dma_start(out[0:64, :], q_h[0][:, 0:SEQ_SH])
            return

        # ================= Phase B: attention =================
        with ExitStack() as bctx:
            epool = bctx.enter_context(tc.tile_pool(name=f"pe{s}", bufs=17))
            npool = bctx.enter_context(tc.tile_pool(name=f"pn{s}", bufs=2))
            o2pool = bctx.enter_context(tc.tile_pool(name=f"po2{s}", bufs=2))
            spool = bctx.enter_context(
                tc.tile_pool(name=f"ps{s}", bufs=2, space="PSUM"))
            pvpool = bctx.enter_context(
                tc.tile_pool(name=f"ppv{s}", bufs=1, space="PSUM"))

            def stage_head(b, h, attn2h):
                # transpose [q, j*64+d] -> stg [jpar*64+d, cb, q] and stage
                # both parities into the per-head bounce (rows sx*64 + d)
                stg = o2pool.tile([128, L // 2], BF16, tag=f"stg{h}",
                                  name=f"stg{b}{h}{s}")
                nc.sync.dma_start(
                    stg[:].rearrange("p (cb r) -> p cb r", r=128),
                    attn2h[:], transpose=True)
                for par in range(2):
                    for cb2 in range(2):
                        dst = bnc_in.rearrange(
                            "(sx e) (cb2 pr r) -> e sx cb2 pr r",
                            e=128, cb2=2, pr=2)[
                            h * 64:(h + 1) * 64, 4 * b:4 * b + 4, cb2, par]
                        src = stg[par * 64:(par + 1) * 64, :].rearrange(
                            "d (sx cb2 r) -> d sx cb2 r",
                            sx=4, cb2=2)[:, :, cb2]
                        nc.sync.dma_start(dst, src)

            attn2h = None
            for b in range(B):
                boff = b * L
                for h in range(2):
                    attn2h = o2pool.tile([128, N_QB * 64], BF16,
                                         name=f"at2{b}{h}{s}", tag=f"at2{h}")
                    # pvT psum: qblocks packed at stride 65, split 7/7/2 so
                    # no matmul output crosses a psum bank.
                    pv_t = [pvpool.tile([128, n * 65], F32, tag=f"pv{i}",
                                        name=f"pv{b}{h}{i}{s}")
                            for i, n in enumerate((7, 7, 2))]

                    def pv_ap(j):
                        t, jj = (0, j) if j < 7 else \
                                (1, j - 7) if j < 14 else (2, j - 14)
                        return pv_t[t][:, jj * 65:(jj + 1) * 65]

                    den_sb = npool.tile([128, N_QB], F32, tag="den",
                                        name=f"den{b}{h}{s}")
                    ex_t = {}          # (ki, sg) -> (tile, qlo)

                    def emit_scores(ki):
                        kcols = k_h[h][:, boff + ki * KT:
                                       boff + (ki + 1) * KT]
                        for sg in range(ki // 8, N_SEG):
                            qlo = max(sg * SEG, ki * KT)
                            qhi = (sg + 1) * SEG
                            sc = spool.tile([128, SEG], F32, tag="sc",
                                            name=f"sc{b}{h}{ki}{sg}{s}")
                            for half in range(2):
                                m0 = max(qlo, sg * SEG + half * 512)
                                m1 = sg * SEG + (half + 1) * 512
                                if m0 < m1:
                                    nc.tensor.matmul(
                                        sc[:, m0 - sg * SEG:m1 - sg * SEG],
                                        kcols,
                                        q_h[h][:, boff + m0:boff + m1],
                                        start=True, stop=True)
                            ex = epool.tile([128, SEG], BF16, tag=f"ex{sg}",
                                            name=f"ex{b}{h}{ki}{sg}{s}")
                            nc.scalar.activation(
                                ex[:, 0:qhi - qlo],
                                sc[:, qlo - sg * SEG:SEG],
                                mybir.ActivationFunctionType.Exp)
                            if qlo == ki * KT:
                                nc.gpsimd.tensor_mul(
                                    ex[:, 0:KT], ex[:, 0:KT], tril_sb[:])
                            ex_t[(ki, sg)] = (ex, qlo)

                    def emit_pv(j):
                        # one sequential psum accumulation group per qblock
                        sg = j // 8
                        for kk in range(j + 1):
                            ex, qlo = ex_t[(kk, sg)]
                            nc.tensor.matmul(
                                pv_ap(j),
                                ex[:, j * KT - qlo:(j + 1) * KT - qlo],
                                va[b][:, kk * 130 + h * 65:
                                      kk * 130 + (h + 1) * 65],
                                start=(kk == 0), stop=(kk == j))
                        if j in (6, 13, 15):
                            t0 = 0 if j == 6 else (7 if j == 13 else 14)
                            n = j - t0 + 1
                            ti = 0 if j == 6 else (1 if j == 13 else 2)
                            nc.vector.reciprocal(
                                den_sb[:, t0:t0 + n],
                                pv_t[ti][:].rearrange(
                                    "p (j w) -> p j w", w=65)[:, :, 64])
                            for jj in range(t0, j + 1):
                                nc.vector.tensor_scalar_mul(
                                    attn2h[:, jj * 64:(jj + 1) * 64],
                                    pv_ap(jj)[:, 0:64],
                                    den_sb[:, jj:jj + 1])

                    for ki in range(N_KT):
                        emit_scores(ki)
                        if ki >= 1:
                            emit_pv(ki - 1)
                    emit_pv(N_KT - 1)
                    stage_head(b, h, attn2h)
            if getattr(nc, "_no_cc", False):
                nc.sync.dma_start(bnc_out[:], bnc_in[:])
            elif getattr(nc, "_stage", "full") == "full":
                nc.gpsimd.collective_compute(
                    "AllToAll", mybir.AluOpType.bypass,
                    replica_groups=[list(range(N_CORES))],
                    ins=[bnc_in[:].opt()],
                    outs=[bnc_out[:].opt()],
                )

            if getattr(nc, "_stage", "full") in ("AB", "ABraw"):
                nc.sync.dma_start(out[0:128, :], attn2h[:, 0:SEQ_SH])
                return

        # ================= Phase C: output projection =================
        with ExitStack() as cctx:
            fpool = cctx.enter_context(tc.tile_pool(name=f"pf{s}", bufs=1))
            fps = cctx.enter_context(
                tc.tile_pool(name=f"pfp{s}", bufs=1, space="PSUM"))

            po = [fps.tile([128, SEQ_SH], F32, tag=f"po{dd}",
                           name=f"po{dd}{s}") for dd in range(D // 128)]
            for half in range(2):
                rbh = fpool.tile([128, 4 * SEQ_SH], BF16,
                                 name=f"rb{half}{s}", tag=f"rb{half}")
                for t in range(4):
                    g = half * 4 + t
                    nc.sync.dma_start(
                        rbh[:, t * SEQ_SH:(t + 1) * SEQ_SH],
                        bnc_out[g * 128:(g + 1) * 128, :])
                for dd in range(D // 128):
                    for t in range(4):
                        g = half * 4 + t
                        nc.tensor.matmul(
                            po[dd][:],
                            wo_all[:, g * D + dd * 128:
                                   g * D + (dd + 1) * 128],
                            rbh[:, t * SEQ_SH:(t + 1) * SEQ_SH],
                            start=(g == 0), stop=(g == 7))
            fo = fpool.tile([128, D // 128 * SEQ_SH], BF16, name=f"fo{s}",
                            tag="fo")
            for dd in range(D // 128):
                nc.vector.tensor_scalar_add(
                    fo[:, dd * SEQ_SH:(dd + 1) * SEQ_SH], po[dd][:],
                    bias_sb[:, dd:dd + 1])
            nc.sync.dma_start(
                out.rearrange("(dd p) c -> p dd c", p=128),
                fo[:].rearrange("p (dd c) -> p dd c", dd=8))


def _host_prep(x, rope_cos, rope_sin, W_qkv, W_out, b_out):
    """Build per-core input maps."""
    import ml_dtypes
    bf = ml_dtypes.bfloat16
    x = np.asarray(x, np.float32)
    rope_cos = np.asarray(rope_cos, np.float32)
    rope_sin = np.asarray(rope_sin, np.float32)
    W_qkv = np.asarray(W_qkv, np.float32)
    W_out = np.asarray(W_out, np.float32)
    b_out = np.asarray(b_out, np.float32)

    xt = np.ascontiguousarray(x.reshape(BL, D).T).astype(bf)     # [D, BL]
    cos_pk = np.ascontiguousarray(
        np.tile(rope_cos[:L].T, (2, 1))).astype(bf)              # [128, L]
    sin_pm = np.ascontiguousarray(
        np.tile(rope_sin[:L].T, (2, 1))).astype(bf)              # [128, L]
    # scoresT layout is [kpos, q]: valid entries have q >= kpos -> upper tri
    tril_m = np.triu(np.ones((KT, KT), np.float32)).astype(bf)
    bias8 = np.ascontiguousarray(b_out.reshape(D // 128, 128).T)
    scale = HD ** -0.5

    # rotate-half permutation: (P q)[i] = -q[i+32] (i<32), q[i-32] (i>=32)
    P = np.zeros((HD, HD), np.float32)
    for i in range(32):
        P[i, i + 32] = -1.0
        P[i + 32, i] = 1.0
    P2 = np.zeros((EV, EV), np.float32)
    P2[:HD, :HD] = P
    P2[HD:, HD:] = P
    p2t_b = np.ascontiguousarray(P2.T).astype(bf)

    wo_h = np.ascontiguousarray(W_out).astype(bf)

    in_maps = []
    for r in range(N_CORES):
        hs = slice(r * H_PER * HD, (r + 1) * H_PER * HD)
        w_q = W_qkv[:, 0:1024][:, hs] * scale
        w_k = W_qkv[:, 1024:2048][:, hs]
        w_v = W_qkv[:, 2048:3072][:, hs]
        w3 = np.ascontiguousarray(np.concatenate(
            [w_q, w_k, w_v], axis=1)).astype(bf)
        in_maps.append({
            "xt": xt, "w3": w3, "p2t": p2t_b, "wo": wo_h,
            "cos_pk": cos_pk, "sin_pm": sin_pm,
            "tril": tril_m, "bias8": bias8,
        })
    return in_maps


_NC_CACHE = {}


def kernel(x, rope_cos, rope_sin, W_qkv, W_out, b_out):
    if "nc" not in _NC_CACHE:
        _NC_CACHE["nc"] = build()
    nc = _NC_CACHE["nc"]
    in_maps = _host_prep(x, rope_cos, rope_sin, W_qkv, W_out, b_out)
    res = run_bass_kernel_spmd(nc, in_maps, core_ids=list(range(N_CORES)))
    outp = np.empty((BL, D), np.float32)
    for r in range(N_CORES):
        outp[r * SEQ_SH:(r + 1) * SEQ_SH, :] = \
            res.results[r]["out"].astype(np.float32).T
    return outp.reshape(B, L, D)


# revision 21
# speedup vs baseline: 4.4390x; 1.0020x over previous
"""Distributed TRN2 Bass kernel for causal multi-head attention
(B=2, L=2048, D=1024, H=16, HD=64) on 8 NeuronCores.

Sharding: tensor-parallel over heads — 2 heads per core, full sequence on
every core. Two 8-core AllToAlls (one per local head) re-shard the per-head
attention outputs by sequence block; core r computes output rows
[r*512, (r+1)*512) of the output projection. The host concatenates the 8
slices. The h0 AllToAll overlaps the h1 attention compute; the h0 half of
the output projection overlaps the h1 AllToAll.

Design notes:
- QKV projection computes q, k, v (3x128 cols). Rotate-half is one extra
  512-col PE matmul per chunk against the block-diagonal permutation P2
  (q,k copied psum->SBUF by the otherwise-idle ACT engine). The rope
  combine is a pure per-chunk dataflow (2 DVE muls + 1 Pool add).
- Everything DMA-fed is bf16 (x, weights, rope tables, q/k/v, exp scores,
  bounce, out). PE runs 1 cycle/row on bf16; FWL keeps LDWEIGHTS off the
  critical path. PSUM accumulation stays fp32.
- PV is computed transposed: stationary = exp-scores tile [kpos, q-block],
  moving = v_aug [kpos, 65] -> psum [q, 64+den]. The softmax denominator
  lands per-partition, so normalization is reciprocal[128,1] +
  tensor_scalar mul — no partition broadcasts. PSUM accumulation groups
  are strictly sequential per bank (start=True clears has_written
  bank-wide, so interleaved groups corrupt each other).
- V and attention-output transposes use the DMA XBAR transpose; its out AP
  must be [p, nblk, rows] with the last dim contiguous — strided variants
  silently corrupt.
- DMA instruction count is kept small: each dma_start costs ~565ns of SP
  sequencer + ~625ns HWDGE generation.
"""
import numpy as np

import concourse.bass as bass
import concourse.tile as tile
from concourse import bacc, mybir
from concourse.bass_utils import run_bass_kernel_spmd

# problem shape (hardcoded per harness contract)
B, L, D = 2, 2048, 1024
H, HD = 16, 64
BL = B * L                      # 4096
N_CORES = 8
H_PER = H // N_CORES            # 2 heads per core
EV = H_PER * HD                 # 128: packed per-core head dim
SEQ_SH = BL // N_CORES          # 512: output rows per core after A2A

F32 = mybir.dt.float32
BF16 = mybir.dt.bfloat16
CHUNK = 512                     # moving-dim chunk for projection
N_CH = BL // CHUNK              # 16 (8 per batch)
KT = 128                        # kpos tile
N_KT = L // KT                  # 16 kpos tiles per batch
SEG = 1024                      # score/exp segment width (2 psum banks)
N_SEG = L // SEG                # 2 per batch
N_QB = L // KT                  # 16 qblocks (128 wide) per batch


def build(dup=1, no_cc=False, stage='full'):
    nc = bacc.Bacc("TRN2", target_bir_lowering=False, debug=False,
                   num_devices=N_CORES)

    xt = nc.dram_tensor("xt", [D, BL], BF16, kind="ExternalInput").ap()
    w3 = nc.dram_tensor("w3", [D, 3 * EV], BF16, kind="ExternalInput").ap()
    p2t = nc.dram_tensor("p2t", [EV, EV], BF16, kind="ExternalInput").ap()
    wo = nc.dram_tensor("wo", [D, D], BF16, kind="ExternalInput").ap()
    cos_pk = nc.dram_tensor("cos_pk", [EV, L], BF16, kind="ExternalInput").ap()
    sin_pm = nc.dram_tensor("sin_pm", [EV, L], BF16, kind="ExternalInput").ap()
    tril = nc.dram_tensor("tril", [KT, KT], BF16, kind="ExternalInput").ap()
    bias8 = nc.dram_tensor("bias8", [128, D // 128], F32,
                           kind="ExternalInput").ap()
    out = nc.dram_tensor("out", [D, SEQ_SH], BF16, kind="ExternalOutput").ap()

    nc._no_cc = no_cc
    nc._stage = stage
    with tile.TileContext(nc) as tc:
        for it in range(dup):
            _emit(nc, tc, it, xt, w3, p2t, wo, cos_pk, sin_pm, tril, bias8,
                  out)
    nc.compile()
    return nc


def _emit(nc, tc, it, xt, w3, p2t, wo, cos_pk, sin_pm, tril, bias8, out):
    from contextlib import ExitStack
    s = f"_{it}"
    # bounce: rows = sx*128 + h*64 + d (d = head-local ev dim)
    bnc_in = nc.dram_tensor(f"bnc_in{s}", [N_CORES * EV, SEQ_SH], BF16)
    bnc_out = nc.dram_tensor(f"bnc_out{s}", [N_CORES * EV, SEQ_SH], BF16)

    def x_load(pool, b, ch):
        c0g = b * L + ch * CHUNK
        xt_big = pool.tile([128, 8 * CHUNK], BF16,
                           name=f"x{b}{ch}{s}", tag="xt")
        nc.sync.dma_start(
            xt_big[:].rearrange("p (c n) -> p c n", c=8),
            xt.rearrange("(c p) n -> p c n", p=128)[:, :, c0g:c0g + CHUNK])
        return xt_big

    with ExitStack() as ctx:
        # ---- persistent pools ----
        cpool = ctx.enter_context(tc.tile_pool(name=f"const{s}", bufs=1))
        qkpool = ctx.enter_context(tc.tile_pool(name=f"qk{s}", bufs=1))
        vpool = ctx.enter_context(tc.tile_pool(name=f"v{s}", bufs=1))

        w3_all = cpool.tile([128, 8 * 3 * EV], BF16, name=f"w3a{s}", tag="w3a")
        nc.sync.dma_start(
            w3_all[:].rearrange("p (c w) -> p c w", c=8),
            w3.rearrange("(c p) w -> p c w", p=128))
        p2_sb = cpool.tile([EV, EV], BF16, name=f"p2{s}", tag="p2")
        nc.sync.dma_start(p2_sb[:], p2t[:])
        wo_all = cpool.tile([128, 8 * D], BF16, name=f"woa{s}", tag="woa")
        cos_sb = cpool.tile([EV, L], BF16, name=f"cos{s}", tag="cos")
        sin_sb = cpool.tile([EV, L], BF16, name=f"sin{s}", tag="sin")
        tril_sb = cpool.tile([KT, KT], BF16, name=f"tril{s}", tag="tril")
        bias_sb = cpool.tile([128, D // 128], F32, name=f"bias{s}", tag="bias")

        # per-head q/k over full sequence
        q_h = [qkpool.tile([64, BL], BF16, name=f"q{h}{s}", tag=f"q{h}")
               for h in range(2)]
        k_h = [qkpool.tile([64, BL], BF16, name=f"k{h}{s}", tag=f"k{h}")
               for h in range(2)]
        # v_aug per batch: 16 kt blocks of (64 v | 1)(64 v | 1), stride 130
        va = [vpool.tile([128, N_KT * 130], BF16, name=f"va{b}{s}",
                         tag=f"va{b}") for b in range(B)]

        # ================= Phase A: projections + rope =================
        with ExitStack() as actx:
            apool = actx.enter_context(tc.tile_pool(name=f"pa{s}", bufs=5))
            rpool = actx.enter_context(tc.tile_pool(name=f"pr{s}", bufs=2))
            ppool = actx.enter_context(
                tc.tile_pool(name=f"pap{s}", bufs=2, space="PSUM"))
            rotpool = actx.enter_context(
                tc.tile_pool(name=f"rpp{s}", bufs=1, space="PSUM"))

            # prime the DMA queue: first x chunks before table loads
            xq = {(0, 0): x_load(apool, 0, 0), (0, 1): x_load(apool, 0, 1)}
            nc.sync.dma_start(cos_sb[:], cos_pk[:])
            nc.sync.dma_start(sin_sb[:], sin_pm[:])
            nc.sync.dma_start(tril_sb[:], tril[:])
            nc.sync.dma_start(bias_sb[:], bias8[:])
            for b in range(B):
                nc.vector.memset(
                    va[b][:].rearrange("p (m w) -> p m w", w=65)[:, :, 64:65],
                    1.0)

            for b in range(B):
                pk = [rpool.tile([128, L], BF16, name=f"pk{qk}{b}{s}",
                                 tag=f"pk{qk}") for qk in range(2)]
                v_sb = rpool.tile([128, L], BF16, name=f"vsb{b}{s}", tag="vsb")
                for ch in range(N_CH // B):
                    lc = ch * CHUNK               # batch-local column
                    xt_big = xq.pop((b, ch), None)
                    if xt_big is None:
                        xt_big = x_load(apool, b, ch)
                    ps3 = ppool.tile([128, 3 * CHUNK], F32, tag="ps3",
                                     name=f"ps3{b}{ch}{s}")
                    for e3 in range(3):
                        for c in range(8):
                            nc.tensor.matmul(
                                ps3[:, e3 * CHUNK:(e3 + 1) * CHUNK],
                                w3_all[:, c * 384 + e3 * 128:
                                       c * 384 + (e3 + 1) * 128],
                                xt_big[:, c * CHUNK:(c + 1) * CHUNK],
                                start=(c == 0), stop=(c == 7))
                    nc.vector.tensor_copy(v_sb[:, lc:lc + CHUNK],
                                          ps3[:, 2 * CHUNK:3 * CHUNK])
                    qsb = apool.tile([128, 2 * CHUNK], BF16, tag="qsb",
                                     name=f"qsb{b}{ch}{s}")
                    nc.scalar.copy(qsb[:], ps3[:, 0:2 * CHUNK])
                    rot = rotpool.tile([128, 2 * CHUNK], F32, tag="rot",
                                       name=f"rot{b}{ch}{s}")
                    for rh in range(2 * CHUNK // 512):
                        nc.tensor.matmul(rot[:, rh * 512:(rh + 1) * 512],
                                         p2_sb[:],
                                         qsb[:, rh * 512:(rh + 1) * 512],
                                         start=True, stop=True)
                    for qk in range(2):
                        t1 = apool.tile([128, CHUNK], F32, tag=f"t1{qk}",
                                        name=f"t1{qk}{b}{ch}{s}")
                        nc.vector.tensor_mul(
                            t1[:], ps3[:, qk * CHUNK:(qk + 1) * CHUNK],
                            cos_sb[:, lc:lc + CHUNK])
                        t2 = apool.tile([128, CHUNK], F32, tag=f"t2{qk}",
                                        name=f"t2{qk}{b}{ch}{s}")
                        nc.vector.tensor_mul(
                            t2[:], rot[:, qk * CHUNK:(qk + 1) * CHUNK],
                            sin_sb[:, lc:lc + CHUNK])
                        nc.gpsimd.tensor_add(pk[qk][:, lc:lc + CHUNK],
                                             t1[:], t2[:])
                # prefetch next batch's first x chunks ahead of tail DMAs
                if b + 1 < B:
                    xq[(b + 1, 0)] = x_load(apool, b + 1, 0)
                    xq[(b + 1, 1)] = x_load(apool, b + 1, 1)
                # batch b done: split heads, transpose v.
                for qk, dsts in enumerate([q_h, k_h]):
                    nc.sync.dma_start(dsts[0][:, b * L:(b + 1) * L],
                                      pk[qk][0:64, :])
                    nc.sync.dma_start(dsts[1][:, b * L:(b + 1) * L],
                                      pk[qk][64:128, :])
                # XBAR transpose needs a contiguous [p, j, rows] out AP;
                # redistribute into the 65-stride va layout with a plain
                # strided SB2SB copy.
                va_j = va[b][:].rearrange("p (j w) -> p j w", w=130)
                for hf in range(2):
                    vh = rpool.tile([128, N_KT * 64], BF16, tag=f"vh{hf}",
                                    name=f"vh{hf}{b}{s}")
                    nc.sync.dma_start(
                        vh[:].rearrange("p (j c) -> p j c", c=64),
                        v_sb[hf * 64:(hf + 1) * 64, :], transpose=True)
                    nc.sync.dma_start(
                        va_j[:, :, hf * 65:hf * 65 + 64],
                        vh[:].rearrange("p (j c) -> p j c", c=64))

        # phase-C weights: load during attention (DMA device is idle then)
        nc.sync.dma_start(
            wo_all[:].rearrange("p (g w) -> p g w", g=8),
            wo.rearrange("(g p) w -> p g w", p=128))

        if getattr(nc, "_stage", "full") == "A":
            nc.sync.dma_start(out[0:64, :], q_h[0][:, 0:SEQ_SH])
            return

        # ================= Phase B: attention =================
        with ExitStack() as bctx:
            epool = bctx.enter_context(tc.tile_pool(name=f"pe{s}", bufs=17))
            npool = bctx.enter_context(tc.tile_pool(name=f"pn{s}", bufs=2))
            o2pool = bctx.enter_context(tc.tile_pool(name=f"po2{s}", bufs=2))
            spool = bctx.enter_context(
                tc.tile_pool(name=f"ps{s}", bufs=2, space="PSUM"))
            pvpool = bctx.enter_context(
                tc.tile_pool(name=f"ppv{s}", bufs=1, space="PSUM"))

            def stage_head(b, h, attn2h):
                # transpose [q, j*64+d] -> stg [jpar*64+d, cb, q] and stage
                # both parities into the per-head bounce (rows sx*64 + d)
                stg = o2pool.tile([128, L // 2], BF16, tag=f"stg{h}",
                                  name=f"stg{b}{h}{s}")
                nc.sync.dma_start(
                    stg[:].rearrange("p (cb r) -> p cb r", r=128),
                    attn2h[:], transpose=True)
                for par in range(2):
                    for cb2 in range(2):
                        dst = bnc_in.rearrange(
                            "(sx e) (cb2 pr r) -> e sx cb2 pr r",
                            e=128, cb2=2, pr=2)[
                            h * 64:(h + 1) * 64, 4 * b:4 * b + 4, cb2, par]
                        src = stg[par * 64:(par + 1) * 64, :].rearrange(
                            "d (sx cb2 r) -> d sx cb2 r",
                            sx=4, cb2=2)[:, :, cb2]
                        nc.sync.dma_start(dst, src)

            attn2h = None
            for b in range(B):
                boff = b * L
                for h in range(2):
                    attn2h = o2pool.tile([128, N_QB * 64], BF16,
                                         name=f"at2{b}{h}{s}", tag=f"at2{h}")
                    # pvT psum: qblocks packed at stride 65, split 7/7/2 so
                    # no matmul output crosses a psum bank.
                    pv_t = [pvpool.tile([128, n * 65], F32, tag=f"pv{i}",
                                        name=f"pv{b}{h}{i}{s}")
                            for i, n in enumerate((7, 7, 2))]

                    def pv_ap(j):
                        t, jj = (0, j) if j < 7 else \
                                (1, j - 7) if j < 14 else (2, j - 14)
                        return pv_t[t][:, jj * 65:(jj + 1) * 65]

                    den_sb = npool.tile([128, N_QB], F32, tag="den",
                                        name=f"den{b}{h}{s}")
                    ex_t = {}          # (ki, sg) -> (tile, qlo)

                    def emit_scores(ki):
                        kcols = k_h[h][:, boff + ki * KT:
                                       boff + (ki + 1) * KT]
                        for sg in range(ki // 8, N_SEG):
                            qlo = max(sg * SEG, ki * KT)
                            qhi = (sg + 1) * SEG
                            sc = spool.tile([128, SEG], F32, tag="sc",
                                            name=f"sc{b}{h}{ki}{sg}{s}")
                            for half in range(2):
                                m0 = max(qlo, sg * SEG + half * 512)
                                m1 = sg * SEG + (half + 1) * 512
                                if m0 < m1:
                                    nc.tensor.matmul(
                                        sc[:, m0 - sg * SEG:m1 - sg * SEG],
                                        kcols,
                                        q_h[h][:, boff + m0:boff + m1],
                                        start=True, stop=True)
                            ex = epool.tile([128, SEG], BF16, tag=f"ex{sg}",
                                            name=f"ex{b}{h}{ki}{sg}{s}")
                            nc.scalar.activation(
                                ex[:, 0:qhi - qlo],
                                sc[:, qlo - sg * SEG:SEG],
                                mybir.ActivationFunctionType.Exp)
                            if qlo == ki * KT:
                                nc.gpsimd.tensor_mul(
                                    ex[:, 0:KT], ex[:, 0:KT], tril_sb[:])
                            ex_t[(ki, sg)] = (ex, qlo)

                    def emit_pv(j):
                        # one sequential psum accumulation group per qblock
                        sg = j // 8
                        for kk in range(j + 1):
                            ex, qlo = ex_t[(kk, sg)]
                            nc.tensor.matmul(
                                pv_ap(j),
                                ex[:, j * KT - qlo:(j + 1) * KT - qlo],
                                va[b][:, kk * 130 + h * 65:
                                      kk * 130 + (h + 1) * 65],
                                start=(kk == 0), stop=(kk == j))
                        if j in (6, 13, 15):
                            t0 = 0 if j == 6 else (7 if j == 13 else 14)
                            n = j - t0 + 1
                            ti = 0 if j == 6 else (1 if j == 13 else 2)
                            nc.vector.reciprocal(
                                den_sb[:, t0:t0 + n],
                                pv_t[ti][:].rearrange(
                                    "p (j w) -> p j w", w=65)[:, :, 64])
                            for jj in range(t0, j + 1):
                                nc.vector.tensor_scalar_mul(
                                    attn2h[:, jj * 64:(jj + 1) * 64],
                                    pv_ap(jj)[:, 0:64],
                                    den_sb[:, jj:jj + 1])

                    for ki in range(N_KT):
                        emit_scores(ki)
                        if ki >= 1:
                            emit_pv(ki - 1)
                    emit_pv(N_KT - 1)
                    stage_head(b, h, attn2h)
            if getattr(nc, "_no_cc", False):
                nc.sync.dma_start(bnc_out[:], bnc_in[:])
            elif getattr(nc, "_stage", "full") == "full":
                nc.gpsimd.collective_compute(
                    "AllToAll", mybir.AluOpType.bypass,
                    replica_groups=[list(range(N_CORES))],
                    ins=[bnc_in[:].opt()],
                    outs=[bnc_out[:].opt()],
                )

            if getattr(nc, "_stage", "full") in ("AB", "ABraw"):
                nc.sync.dma_start(out[0:128, :], attn2h[:, 0:SEQ_SH])
                return

        # ================= Phase C: output projection =================
        with ExitStack() as cctx:
            fpool = cctx.enter_context(tc.tile_pool(name=f"pf{s}", bufs=1))
            fps = cctx.enter_context(
                tc.tile_pool(name=f"pfp{s}", bufs=1, space="PSUM"))

            po = [fps.tile([128, SEQ_SH], F32, tag=f"po{dd}",
                           name=f"po{dd}{s}") for dd in range(D // 128)]
            for half in range(2):
                rbh = fpool.tile([128, 4 * SEQ_SH], BF16,
                                 name=f"rb{half}{s}", tag=f"rb{half}")
                for t in range(4):
                    g = half * 4 + t
                    nc.sync.dma_start(
                        rbh[:, t * SEQ_SH:(t + 1) * SEQ_SH],
                        bnc_out[g * 128:(g + 1) * 128, :])
                for dd in range(D // 128):
                    for t in range(4):
                        g = half * 4 + t
                        nc.tensor.matmul(
                            po[dd][:],
                            wo_all[:, g * D + dd * 128:
                                   g * D + (dd + 1) * 128],
                            rbh[:, t * SEQ_SH:(t + 1) * SEQ_SH],
                            start=(g == 0), stop=(g == 7))
            fo = fpool.tile([128, D // 128 * SEQ_SH], BF16, name=f"fo{s}",
                            tag="fo")
            for half in range(2):
                for dd in range(half * 4, half * 4 + 4):
                    nc.vector.tensor_scalar_add(
                        fo[:, dd * SEQ_SH:(dd + 1) * SEQ_SH], po[dd][:],
                        bias_sb[:, dd:dd + 1])
                nc.sync.dma_start(
                    out.rearrange("(dd p) c -> p dd c",
                                  p=128)[:, half * 4:half * 4 + 4],
                    fo[:].rearrange("p (dd c) -> p dd c",
                                    dd=8)[:, half * 4:half * 4 + 4])


def _host_prep(x, rope_cos, rope_sin, W_qkv, W_out, b_out):
    """Build per-core input maps."""
    import ml_dtypes
    bf = ml_dtypes.bfloat16
    x = np.asarray(x, np.float32)
    rope_cos = np.asarray(rope_cos, np.float32)
    rope_sin = np.asarray(rope_sin, np.float32)
    W_qkv = np.asarray(W_qkv, np.float32)
    W_out = np.asarray(W_out, np.float32)
    b_out = np.asarray(b_out, np.float32)

    xt = np.ascontiguousarray(x.reshape(BL, D).T).astype(bf)     # [D, BL]
    cos_pk = np.ascontiguousarray(
        np.tile(rope_cos[:L].T, (2, 1))).astype(bf)              # [128, L]
    sin_pm = np.ascontiguousarray(
        np.tile(rope_sin[:L].T, (2, 1))).astype(bf)              # [128, L]
    # scoresT layout is [kpos, q]: valid entries have q >= kpos -> upper tri
    tril_m = np.triu(np.ones((KT, KT), np.float32)).astype(bf)
    bias8 = np.ascontiguousarray(b_out.reshape(D // 128, 128).T)
    scale = HD ** -0.5

    # rotate-half permutation: (P q)[i] = -q[i+32] (i<32), q[i-32] (i>=32)
    P = np.zeros((HD, HD), np.float32)
    for i in range(32):
        P[i, i + 32] = -1.0
        P[i + 32, i] = 1.0
    P2 = np.zeros((EV, EV), np.float32)
    P2[:HD, :HD] = P
    P2[HD:, HD:] = P
    p2t_b = np.ascontiguousarray(P2.T).astype(bf)

    wo_h = np.ascontiguousarray(W_out).astype(bf)

    in_maps = []
    for r in range(N_CORES):
        hs = slice(r * H_PER * HD, (r + 1) * H_PER * HD)
        w_q = W_qkv[:, 0:1024][:, hs] * scale
        w_k = W_qkv[:, 1024:2048][:, hs]
        w_v = W_qkv[:, 2048:3072][:, hs]
        w3 = np.ascontiguousarray(np.concatenate(
            [w_q, w_k, w_v], axis=1)).astype(bf)
        in_maps.append({
            "xt": xt, "w3": w3, "p2t": p2t_b, "wo": wo_h,
            "cos_pk": cos_pk, "sin_pm": sin_pm,
            "tril": tril_m, "bias8": bias8,
        })
    return in_maps


_NC_CACHE = {}


def kernel(x, rope_cos, rope_sin, W_qkv, W_out, b_out):
    if "nc" not in _NC_CACHE:
        _NC_CACHE["nc"] = build()
    nc = _NC_CACHE["nc"]
    in_maps = _host_prep(x, rope_cos, rope_sin, W_qkv, W_out, b_out)
    res = run_bass_kernel_spmd(nc, in_maps, core_ids=list(range(N_CORES)))
    outp = np.empty((BL, D), np.float32)
    for r in range(N_CORES):
        outp[r * SEQ_SH:(r + 1) * SEQ_SH, :] = \
            res.results[r]["out"].astype(np.float32).T
    return outp.reshape(B, L, D)
